# revision 5
# baseline (speedup 1.0000x reference)
"""DEQ MLP — Trainium2 Bass kernel, v3 (Picard + folded weights + fp8).

Problem: z* = fixpoint of f(z) = relu(z@W1+b1)@W2+b2, z0 = x@W_in+b_in,
out = z*@W_out + b_out.  B=1024, D=1024.  Reference solves with Anderson
acceleration (m=6, 40 iters); but f is strongly contractive (~0.17/iter),
so plain Picard iteration reaches the bf16-precision fixed point in ~7
steps — no Anderson machinery needed.

Key restructurings (validated numerically against an fp64 oracle):
 1. h-space iteration with host-folded weights: substituting z = h@W2+b2
    into h' = relu(z@W1+b1) gives  h' = relu(h@(W2@W1) + (b2@W1+b1)) —
    ONE 1024x1024 matmul per iteration instead of two.  Input/output
    projections fold likewise.
 2. fp8(e4m3) DoubleRow matmuls (K=256/instr, 0.5 PE cycles/row) for the
    first N_FP8 iterations, power-of-2 scales (W*2^11, act*2^5); an
    optional bf16 polish iteration pins the fixed point to the bf16
    floor.  Emulated end-to-end: (6,1) rel err 2.8e-3, (6,0) 4.4e-3
    (gate 2e-2); HW matched emulation within 4%.
 3. Biases ride IN the matmul as an extra contraction pair (hi/lo fp8
    split on partition 0 x ones vector), so PSUM evictions are pure
    relu(psum * 2^-k).
 4. h lives in FOUR separate 2-chunk tiles (one per DoubleRow pair) so
    the 4 eviction instructions (alternating ACT/DVE) have no
    write-after-write ordering between them — they pipeline under the
    PE's next groups instead of serializing (+1.2us/iter otherwise).
 5. Weight DMAs spread across 4 engine queues (sync/gpsimd/vector/
    scalar), big tensors split in half, so the serial-DMA head shrinks
    from ~17us to ~2us of exposed latency.
 6. Pure data parallel: batch 1024 -> 128 rows/core on 8 cores; weights
    replicated; no collectives; all layouts feature-major, zero device
    transposes.
"""

import os
import sys

for _p in ("/opt/trn_rl_repo", "/root/.axon_site/_ro/trn_rl_repo"):
    if os.path.isdir(_p) and _p not in sys.path:
        sys.path.insert(0, _p)

import numpy as np
import ml_dtypes

import concourse.bass as bass
import concourse.mybir as mybir
from concourse.tile import TileContext

BF16 = mybir.dt.bfloat16
FP8 = mybir.dt.float8e4
F32 = mybir.dt.float32
AL = mybir.AluOpType
AF = mybir.ActivationFunctionType
DR = mybir.MatmulPerfMode.DoubleRow

P = 128
D = 1024           # hidden width (h space)
DIN = 512
DOUT = 512
NCD = D // P       # 8
NCI = DIN // P     # 4
NCO = DOUT // P    # 4
NPAIR = NCD // 2   # 4 DoubleRow pairs over the hidden dim
N_CORES = 8
B = 1024 // N_CORES  # 128 batch rows per core

# power-of-2 scales for fp8: weights *2^11, activations *2^5, psum *2^16
SWL, SZL = 11, 5
SPL = SWL + SZL

N_FP8 = 6          # fp8 Picard iterations
N_BF16 = 1         # bf16 polish iterations

bf16 = ml_dtypes.bfloat16
fp8 = ml_dtypes.float8_e4m3


def _emit(nc: bass.Bass, tc, ctx, n8: int, nb: int):
    # ---------------- DRAM I/O ----------------
    def din(name, free, dt):
        return nc.declare_dram_parameter(name, [P, free], dt, isOutput=False)

    d_x8 = din("x8", NCI * B, FP8)
    d_ones8 = din("ones8", 2 * B, FP8)
    d_onesb = din("onesb", B, BF16)
    d_win18 = din("win18", 2 * 2 * NCD * P, FP8)
    d_cin8 = din("cin8", 2 * NCD * P, FP8)
    d_w218 = din("w218", 4 * 2 * NCD * P, FP8)
    d_c8 = din("c8", 2 * NCD * P, FP8)
    if nb > 0:
        d_w21b = din("w21b", NCD * NCD * P, BF16)
        d_cb = din("cb", NCD * P, BF16)
    d_w2outb = din("w2outb", NCD * NCO * P, BF16)
    d_coutb = din("coutb", NCO * P, BF16)
    d_out = nc.declare_dram_parameter("out", [P, NCO * B], F32, isOutput=True)

    consts = ctx.enter_context(tc.tile_pool(name="consts", bufs=1))
    state = ctx.enter_context(tc.tile_pool(name="state", bufs=1))
    # one pool per DoubleRow pair so the 4 evictions have no WAW ordering
    h8ps = [ctx.enter_context(tc.tile_pool(name=f"h8p{j}", bufs=2))
            for j in range(NPAIR)]
    hbps = [ctx.enter_context(tc.tile_pool(name=f"hbp{j}", bufs=2))
            for j in range(NPAIR)]
    otps = [ctx.enter_context(tc.tile_pool(name=f"otp{j}", bufs=1))
            for j in range(2)]
    pp = ctx.enter_context(tc.tile_pool(name="pp", bufs=2, space="PSUM"))
    op = ctx.enter_context(tc.tile_pool(name="op", bufs=1, space="PSUM"))

    # ---------------- constants into SBUF ----------------
    x8 = consts.tile([P, NCI, B], FP8)
    ones8 = consts.tile([P, 2, B], FP8)
    onesb = consts.tile([P, B], BF16)
    Win18 = consts.tile([P, 2, 2, NCD, P], FP8)
    Cin8 = consts.tile([P, 2, NCD, P], FP8)
    W218 = consts.tile([P, 4, 2, NCD, P], FP8)
    C8 = consts.tile([P, 2, NCD, P], FP8)
    if nb > 0:
        W21b = consts.tile([P, NCD, NCD, P], BF16)
        Cb = consts.tile([P, NCD, P], BF16)
    W2outb = consts.tile([P, NCD, NCO, P], BF16)
    Coutb = consts.tile([P, NCO, P], BF16)

    # DMA queue assignment (only SP/Activation/gpsimd may issue DMAs):
    # three parallel queues, criticals first on each.
    nc.sync.dma_start(out=x8[:, :, :], in_=d_x8[:, :])
    nc.sync.dma_start(out=ones8[:, :, :], in_=d_ones8[:, :])
    nc.sync.dma_start(out=Cin8[:, :, :, :], in_=d_cin8[:, :])
    nc.gpsimd.dma_start(out=Win18[:, :, :, :, :], in_=d_win18[:, :])
    HP = 4 * 2 * NCD * P // 2
    nc.scalar.dma_start(out=W218[:, 0:2, :, :, :], in_=d_w218[:, 0:HP])
    nc.sync.dma_start(out=W218[:, 2:4, :, :, :], in_=d_w218[:, HP:])
    nc.gpsimd.dma_start(out=C8[:, :, :, :], in_=d_c8[:, :])
    nc.sync.dma_start(out=onesb[:, :], in_=d_onesb[:, :])
    if nb > 0:
        WP = NCD * NCD * P // 2
        nc.scalar.dma_start(out=W21b[:, 0:NCD // 2, :, :], in_=d_w21b[:, 0:WP])
        nc.gpsimd.dma_start(out=W21b[:, NCD // 2 :, :, :], in_=d_w21b[:, WP:])
        nc.sync.dma_start(out=Cb[:, :, :], in_=d_cb[:, :])
    nc.gpsimd.dma_start(out=W2outb[:, :, :, :], in_=d_w2outb[:, :])
    nc.sync.dma_start(out=Coutb[:, :, :], in_=d_coutb[:, :])

    def evict4(pt, out_tiles, scale, relu):
        """PSUM -> 4 separate 2-chunk tiles, alternating ACT/DVE."""
        for j in range(NPAIR):
            sl = slice(2 * j, 2 * j + 2)
            if j % 2 == 0:
                nc.scalar.activation(
                    out_tiles[j][:, :, :], pt[:, sl, :],
                    AF.Relu if relu else AF.Copy, bias=0.0, scale=scale,
                )
            else:
                nc.vector.tensor_scalar(
                    out=out_tiles[j][:, :, :], in0=pt[:, sl, :],
                    scalar1=scale, scalar2=0.0, op0=AL.mult, op1=AL.max,
                )

    def fp8_layer(Wt, biast, rhs_pairs, npairs, out_tiles, out_scale):
        pt = pp.tile([P, NCD, B], F32)
        for n in range(NCD):
            nc.tensor.matmul(pt[:, n, :], lhsT=biast[:, :, n, :],
                             rhs=ones8[:, :, :], start=True, stop=False,
                             perf_mode=DR)
            for cp in range(npairs):
                nc.tensor.matmul(pt[:, n, :], lhsT=Wt[:, cp, :, n, :],
                                 rhs=rhs_pairs[cp], start=False,
                                 stop=(cp == npairs - 1), perf_mode=DR)
        evict4(pt, out_tiles, out_scale, relu=True)

    _tc = [0]

    def new_tiles(pools, dt):
        _tc[0] += 1
        return [pool.tile([P, 2, B], dt, name=f"h{_tc[0]}_{j}")
                for j, pool in enumerate(pools)]

    S8 = 2.0 ** (SZL - SPL)    # psum -> fp8-scaled h
    SB = 2.0 ** (-SPL)         # psum -> unscaled bf16 h

    # ---------------- program ----------------
    # in-proj: h = relu(x @ W_in1 + c_in), fp8, K=512 (2 pairs)
    h8 = new_tiles(h8ps, FP8)
    xp = [x8[:, 0:2, :], x8[:, 2:4, :]]
    fp8_layer(Win18, Cin8, xp, 2, h8, S8)

    # fp8 Picard iterations (the last evicts to bf16 for the polish/out)
    hb = None
    for i in range(n8):
        last = i == n8 - 1
        out_tiles = new_tiles(hbps, BF16) if last else new_tiles(h8ps, FP8)
        rhs_pairs = [t[:, :, :] for t in h8]
        fp8_layer(W218, C8, rhs_pairs, 4, out_tiles, SB if last else S8)
        if last:
            hb = out_tiles
        else:
            h8 = out_tiles

    # bf16 polish iterations
    for j in range(nb):
        nxt = new_tiles(hbps, BF16)
        pt = pp.tile([P, NCD, B], F32)
        for n in range(NCD):
            nc.tensor.matmul(pt[:, n, :], lhsT=Cb[:, n, :], rhs=onesb[:, :],
                             start=True, stop=False)
            for c in range(NCD):
                nc.tensor.matmul(pt[:, n, :], lhsT=W21b[:, c, n, :],
                                 rhs=hb[c // 2][:, c % 2, :], start=False,
                                 stop=(c == NCD - 1))
        evict4(pt, nxt, 1.0, relu=True)
        hb = nxt

    # out-proj: out = h @ W2out + c_out  (bf16 weights, fp32 out)
    ot = op.tile([P, NCO, B], F32)
    for o in range(NCO):
        nc.tensor.matmul(ot[:, o, :], lhsT=Coutb[:, o, :], rhs=onesb[:, :],
                         start=True, stop=False)
        for c in range(NCD):
            nc.tensor.matmul(ot[:, o, :], lhsT=W2outb[:, c, o, :],
                             rhs=hb[c // 2][:, c % 2, :], start=False,
                             stop=(c == NCD - 1))
    outT = [pool.tile([P, 2, B], F32, name=f"outT{j}")
            for j, pool in enumerate(otps)]
    nc.scalar.activation(outT[0][:, :, :], ot[:, 0:2, :], AF.Copy,
                         bias=0.0, scale=1.0)
    nc.vector.tensor_scalar(out=outT[1][:, :, :], in0=ot[:, 2:4, :],
                            scalar1=1.0, scalar2=None, op0=AL.mult)
    nc.sync.dma_start(out=d_out[:, 0 : 2 * B], in_=outT[0][:, :, :])
    nc.gpsimd.dma_start(out=d_out[:, 2 * B :], in_=outT[1][:, :, :])


def build_program(n8: int = N_FP8, nb: int = N_BF16) -> bass.Bass:
    from contextlib import ExitStack

    from concourse import bacc

    nc = bacc.Bacc(trn_type="TRN2", target_bir_lowering=False)
    with ExitStack() as ctx:
        tc = ctx.enter_context(TileContext(nc))
        _emit(nc, tc, ctx, n8, nb)
    nc.compile()
    return nc


def _fold_weights(inputs):
    """Host-side weight folding in fp64 (cheap: ~3.5 GFLOP once per call)."""
    f64 = np.float64
    W_in, b_in = inputs["W_in"].astype(f64), inputs["b_in"].astype(f64)
    W1, b1 = inputs["W1"].astype(f64), inputs["b1"].astype(f64)
    W2, b2 = inputs["W2"].astype(f64), inputs["b2"].astype(f64)
    W_out, b_out = inputs["W_out"].astype(f64), inputs["b_out"].astype(f64)
    return {
        "W_in1": W_in @ W1, "c_in": b_in @ W1 + b1,
        "W21": W2 @ W1, "c": b2 @ W1 + b1,
        "W2out": W2 @ W_out, "c_out": b2 @ W_out + b_out,
    }


def _pack_w8(W, sw):
    """[K, N] -> [128, K//256, 2, N//128, 128] fp8 (DoubleRow pairs)."""
    K, N = W.shape
    t = (W * sw).astype(fp8)
    t = t.reshape(K // 256, 2, P, N // P, P).transpose(2, 0, 1, 3, 4)
    return np.ascontiguousarray(t.reshape(P, -1))


def _pack_wb(W):
    """[K, N] -> [128, K//128, N//128, 128] bf16."""
    K, N = W.shape
    t = W.astype(bf16).reshape(K // P, P, N // P, P).transpose(1, 0, 2, 3)
    return np.ascontiguousarray(t.reshape(P, -1))


def _pack_bias8(c):
    """[N] -> [128, 2, N//128, 128] fp8: partition 0 = (hi, lo) rows.
    Paired with the (2^7, 2^4) ones vector this contributes c*2^16."""
    hi64 = c * 2.0 ** (SPL - 7)
    hi = hi64.astype(fp8)
    lo = ((hi64 - hi.astype(np.float64)) * 2.0 ** 3).astype(fp8)
    arr = np.zeros((P, 2, c.shape[0] // P, P), fp8)
    arr[0, 0] = hi.reshape(-1, P)
    arr[0, 1] = lo.reshape(-1, P)
    return np.ascontiguousarray(arr.reshape(P, -1))


def _pack_biasb(c):
    arr = np.zeros((P, c.shape[0] // P, P), bf16)
    arr[0] = c.astype(bf16).reshape(-1, P)
    return np.ascontiguousarray(arr.reshape(P, -1))


def _prep_inputs(inputs, nb: int = N_BF16):
    F = _fold_weights(inputs)
    ones8 = np.zeros((P, 2, B), fp8)
    ones8[0, 0, :] = 2.0 ** 7
    ones8[0, 1, :] = 2.0 ** 4
    onesb = np.zeros((P, B), bf16)
    onesb[0, :] = 1.0
    sw = 2.0 ** SWL
    shared = {
        "ones8": np.ascontiguousarray(ones8.reshape(P, -1)),
        "onesb": onesb,
        "win18": _pack_w8(F["W_in1"], sw),
        "cin8": _pack_bias8(F["c_in"]),
        "w218": _pack_w8(F["W21"], sw),
        "c8": _pack_bias8(F["c"]),
        "w2outb": _pack_wb(F["W2out"]),
        "coutb": _pack_biasb(F["c_out"]),
    }
    if nb > 0:
        shared["w21b"] = _pack_wb(F["W21"])
        shared["cb"] = _pack_biasb(F["c"])
    x = inputs["x"]
    in_maps = []
    for cidx in range(N_CORES):
        xs = x[cidx * B : (cidx + 1) * B].astype(np.float64)   # [128, 512]
        x8t = (xs.T * 2.0 ** SZL).astype(fp8)                  # [512, 128]
        x8t = x8t.reshape(NCI, P, B).transpose(1, 0, 2)        # [128, 4, 128]
        im = {"x8": np.ascontiguousarray(x8t.reshape(P, -1))}
        im.update(shared)
        in_maps.append(im)
    return in_maps


_CACHE = {}


def run_on_hw(inputs, n8: int = N_FP8, nb: int = N_BF16, trace: bool = False):
    """Returns (output [1024, 512] fp32, BassKernelResults)."""
    from concourse.bass_utils import run_bass_kernel_spmd

    key = (n8, nb)
    if key not in _CACHE:
        _CACHE[key] = build_program(n8, nb)
    nc = _CACHE[key]
    in_maps = _prep_inputs(inputs, nb)
    res = run_bass_kernel_spmd(nc, in_maps, list(range(N_CORES)), trace=trace)
    outs = []
    for i in range(N_CORES):
        oT = np.asarray(res.results[i]["out"], dtype=np.float32)  # [128, 4*128]
        oT = oT.reshape(P, NCO, B).transpose(2, 1, 0).reshape(B, DOUT)
        outs.append(oT)
    return np.concatenate(outs, axis=0), res


def bench_on_hw(inputs, n8: int = N_FP8, nb: int = N_BF16, reps: int = 32):
    """Per-execution device time via pipelined repeated execution."""
    import time

    import jax
    from jax.sharding import Mesh, PartitionSpec
    from jax.experimental.shard_map import shard_map

    from concourse import bass2jax, mybir as mb

    key = (n8, nb)
    if key not in _CACHE:
        _CACHE[key] = build_program(n8, nb)
    nc = _CACHE[key]
    bass2jax.install_neuronx_cc_hook()

    partition_name = nc.partition_id_tensor.name if nc.partition_id_tensor else None
    in_names, out_names, out_avals, zero_outs = [], [], [], []
    for alloc in nc.m.functions[0].allocations:
        if not isinstance(alloc, mb.MemoryLocationSet):
            continue
        name = alloc.memorylocations[0].name
        if alloc.kind == "ExternalInput":
            if name != partition_name:
                in_names.append(name)
        elif alloc.kind == "ExternalOutput":
            out_names.append(name)
            shape = tuple(alloc.tensor_shape)
            dtype = mb.dt.np(alloc.dtype)
            out_avals.append(jax.core.ShapedArray(shape, dtype))
            zero_outs.append(np.zeros(shape, dtype))
    n_params = len(in_names)
    in_names_all = in_names + out_names
    if partition_name is not None:
        in_names_all.append(partition_name)

    def _body(*args):
        operands = list(args)
        if partition_name is not None:
            operands.append(bass2jax.partition_id_tensor())
        outs = bass2jax._bass_exec_p.bind(
            *operands,
            out_avals=tuple(out_avals),
            in_names=tuple(in_names_all),
            out_names=tuple(out_names),
            lowering_input_output_aliases=(),
            sim_require_finite=True,
            sim_require_nnan=True,
            nc=nc,
        )
        return tuple(outs)

    in_maps = _prep_inputs(inputs, nb)
    devices = jax.devices()[:N_CORES]
    mesh = Mesh(np.asarray(devices), ("core",))
    in_specs = (PartitionSpec("core"),) * (n_params + len(out_names))
    out_specs = (PartitionSpec("core"),) * len(out_names)
    sharded = jax.jit(
        shard_map(_body, mesh=mesh, in_specs=in_specs, out_specs=out_specs,
                  check_rep=False),
        keep_unused=True,
    )
    concat_in = [
        np.concatenate([np.asarray(in_maps[c][nm]) for c in range(N_CORES)], axis=0)
        for nm in in_names
    ]
    concat_zeros = [
        np.zeros((N_CORES * z.shape[0], *z.shape[1:]), z.dtype) for z in zero_outs
    ]
    args = [jax.device_put(a) for a in concat_in + concat_zeros]
    out = sharded(*args)
    jax.block_until_ready(out)
    best = float("inf")
    for _ in range(3):
        t0 = time.perf_counter()
        outs = [sharded(*args) for _ in range(reps)]
        jax.block_until_ready(outs)
        dt = (time.perf_counter() - t0) / reps
        best = min(best, dt)
    out_np = np.asarray(out[0], dtype=np.float32)
    return best, out_np


def kernel(**inputs) -> np.ndarray:
    out, _ = run_on_hw(inputs)
    return out


if __name__ == "__main__":
    nc = build_program()
    print("built ok")


# revision 8
# speedup vs baseline: 1.7669x; 1.7669x over previous
"""DEQ MLP — Trainium2 Bass kernel, v3 (Picard + folded weights + fp8).

Problem: z* = fixpoint of f(z) = relu(z@W1+b1)@W2+b2, z0 = x@W_in+b_in,
out = z*@W_out + b_out.  B=1024, D=1024.  Reference solves with Anderson
acceleration (m=6, 40 iters); but f is strongly contractive (~0.17/iter),
so plain Picard iteration reaches the bf16-precision fixed point in ~7
steps — no Anderson machinery needed.

Key restructurings (validated numerically against an fp64 oracle):
 1. h-space iteration with host-folded weights: substituting z = h@W2+b2
    into h' = relu(z@W1+b1) gives  h' = relu(h@(W2@W1) + (b2@W1+b1)) —
    ONE 1024x1024 matmul per iteration instead of two.  Input/output
    projections fold likewise.
 2. fp8(e4m3) DoubleRow matmuls (K=256/instr, 0.5 PE cycles/row) for the
    first N_FP8 iterations, power-of-2 scales (W*2^11, act*2^5); an
    optional bf16 polish iteration pins the fixed point to the bf16
    floor.  Emulated end-to-end: (6,1) rel err 2.8e-3, (6,0) 4.4e-3
    (gate 2e-2); HW matched emulation within 4%.
 3. Biases ride IN the matmul as an extra contraction pair (hi/lo fp8
    split on partition 0 x ones vector), so PSUM evictions are pure
    relu(psum * 2^-k).
 4. h lives in FOUR separate 2-chunk tiles (one per DoubleRow pair) so
    the 4 eviction instructions (alternating ACT/DVE) have no
    write-after-write ordering between them — they pipeline under the
    PE's next groups instead of serializing (+1.2us/iter otherwise).
 5. Weight DMAs spread across 4 engine queues (sync/gpsimd/vector/
    scalar), big tensors split in half, so the serial-DMA head shrinks
    from ~17us to ~2us of exposed latency.
 6. Pure data parallel: batch 1024 -> 128 rows/core on 8 cores; weights
    replicated; no collectives; all layouts feature-major, zero device
    transposes.
"""

import os
import sys

for _p in ("/opt/trn_rl_repo", "/root/.axon_site/_ro/trn_rl_repo"):
    if os.path.isdir(_p) and _p not in sys.path:
        sys.path.insert(0, _p)

import numpy as np
import ml_dtypes

import concourse.bass as bass
import concourse.mybir as mybir
from concourse.tile import TileContext

BF16 = mybir.dt.bfloat16
FP8 = mybir.dt.float8e4
F32 = mybir.dt.float32
AL = mybir.AluOpType
AF = mybir.ActivationFunctionType
DR = mybir.MatmulPerfMode.DoubleRow

P = 128
D = 1024           # hidden width (h space)
DIN = 512
DOUT = 512
NCD = D // P       # 8
NCI = DIN // P     # 4
NCO = DOUT // P    # 4
NPAIR = NCD // 2   # 4 DoubleRow pairs over the hidden dim
N_CORES = 8
B = 1024 // N_CORES  # 128 batch rows per core

# power-of-2 scales for fp8: weights *2^11, activations *2^5, psum *2^16
SWL, SZL = 11, 5
SPL = SWL + SZL

N_FP8 = 7          # fp8 Picard iterations
N_BF16 = 0         # bf16 polish iterations (fp8-only hits 4.2e-3; gate 2e-2)

bf16 = ml_dtypes.bfloat16
fp8 = ml_dtypes.float8_e4m3


def _emit(nc: bass.Bass, tc, ctx, n8: int, nb: int):
    # ---------------- DRAM I/O ----------------
    def din(name, free, dt):
        return nc.declare_dram_parameter(name, [P, free], dt, isOutput=False)

    d_x8 = din("x8", NCI * B, FP8)
    d_ones8 = din("ones8", 2 * B, FP8)
    d_onesb = din("onesb", B, BF16)
    d_win18 = din("win18", 2 * 2 * NCD * P, FP8)
    d_cin8 = din("cin8", 2 * NCD * P, FP8)
    d_w218 = din("w218", 4 * 2 * NCD * P, FP8)
    d_c8 = din("c8", 2 * NCD * P, FP8)
    if nb > 0:
        d_w21b = din("w21b", NCD * NCD * P, BF16)
        d_cb = din("cb", NCD * P, BF16)
    d_w2outb = din("w2outb", NCD * NCO * P, BF16)
    d_coutb = din("coutb", NCO * P, BF16)
    d_out = nc.declare_dram_parameter("out", [P, NCO * B], F32, isOutput=True)

    consts = ctx.enter_context(tc.tile_pool(name="consts", bufs=1))
    state = ctx.enter_context(tc.tile_pool(name="state", bufs=1))
    # one pool per DoubleRow pair so the 4 evictions have no WAW ordering
    h8ps = [ctx.enter_context(tc.tile_pool(name=f"h8p{j}", bufs=2))
            for j in range(NPAIR)]
    hbps = [ctx.enter_context(tc.tile_pool(name=f"hbp{j}", bufs=2))
            for j in range(NPAIR)]
    otps = [ctx.enter_context(tc.tile_pool(name=f"otp{j}", bufs=1))
            for j in range(2)]
    pps = [ctx.enter_context(tc.tile_pool(name=f"pp{j}", bufs=2, space="PSUM"))
           for j in range(NPAIR)]

    # ---------------- constants into SBUF ----------------
    x8 = consts.tile([P, NCI, B], FP8)
    ones8 = consts.tile([P, 2, B], FP8)
    onesb = consts.tile([P, B], BF16)
    Win18 = consts.tile([P, 2, 2, NCD, P], FP8)
    Cin8 = consts.tile([P, 2, NCD, P], FP8)
    W218 = consts.tile([P, 4, 2, NCD, P], FP8)
    C8 = consts.tile([P, 2, NCD, P], FP8)
    if nb > 0:
        W21b = consts.tile([P, NCD, NCD, P], BF16)
        Cb = consts.tile([P, NCD, P], BF16)
    W2outb = consts.tile([P, NCD, NCO, P], BF16)
    Coutb = consts.tile([P, NCO, P], BF16)

    # DMA queue assignment (only SP/Activation/gpsimd may issue DMAs):
    # three parallel queues, criticals first on each.
    nc.sync.dma_start(out=x8[:, :, :], in_=d_x8[:, :])
    nc.sync.dma_start(out=ones8[:, :, :], in_=d_ones8[:, :])
    nc.sync.dma_start(out=onesb[:, :], in_=d_onesb[:, :])
    nc.gpsimd.dma_start(out=Cin8[:, :, :, :], in_=d_cin8[:, :])
    nc.scalar.dma_start(out=Win18[:, :, :, :, :], in_=d_win18[:, :])
    HP = 4 * 2 * NCD * P // 2
    nc.scalar.dma_start(out=W218[:, 0:2, :, :, :], in_=d_w218[:, 0:HP])
    nc.sync.dma_start(out=W218[:, 2:4, :, :, :], in_=d_w218[:, HP:])
    nc.gpsimd.dma_start(out=C8[:, :, :, :], in_=d_c8[:, :])
    nc.gpsimd.dma_start(out=W2outb[:, :, :, :], in_=d_w2outb[:, :])
    nc.sync.dma_start(out=Coutb[:, :, :], in_=d_coutb[:, :])
    if nb > 0:
        WP = NCD * NCD * P // 2
        nc.scalar.dma_start(out=W21b[:, 0:NCD // 2, :, :], in_=d_w21b[:, 0:WP])
        nc.gpsimd.dma_start(out=W21b[:, NCD // 2 :, :, :], in_=d_w21b[:, WP:])
        nc.sync.dma_start(out=Cb[:, :, :], in_=d_cb[:, :])

    def evict4(pts, out_tiles, scale, relu):
        """4 independent PSUM tiles -> 4 h tiles, alternating ACT/DVE."""
        for j in range(NPAIR):
            if j % 2 == 0:
                nc.scalar.activation(
                    out_tiles[j][:, :, :], pts[j][:, :, :],
                    AF.Relu if relu else AF.Copy, bias=0.0, scale=scale,
                )
            else:
                nc.vector.tensor_scalar(
                    out=out_tiles[j][:, :, :], in0=pts[j][:, :, :],
                    scalar1=scale, scalar2=0.0, op0=AL.mult, op1=AL.max,
                )

    _pc = [0]

    def new_psums(dt=F32):
        _pc[0] += 1
        return [pool.tile([P, 2, B], dt, name=f"pt{_pc[0]}_{j}", tag=f"pt{j}")
                for j, pool in enumerate(pps)]

    def fp8_layer(Wt, biast, rhs_pairs, npairs, out_tiles, out_scale):
        pts = new_psums()
        for n in range(NCD):
            pslice = pts[n // 2][:, n % 2, :]
            nc.tensor.matmul(pslice, lhsT=biast[:, :, n, :],
                             rhs=ones8[:, :, :], start=True, stop=False,
                             perf_mode=DR)
            for cp in range(npairs):
                nc.tensor.matmul(pslice, lhsT=Wt[:, cp, :, n, :],
                                 rhs=rhs_pairs[cp], start=False,
                                 stop=(cp == npairs - 1), perf_mode=DR)
        evict4(pts, out_tiles, out_scale, relu=True)

    _tc = [0]

    def new_tiles(pools, dt):
        _tc[0] += 1
        return [pool.tile([P, 2, B], dt, name=f"h{_tc[0]}_{j}")
                for j, pool in enumerate(pools)]

    S8 = 2.0 ** (SZL - SPL)    # psum -> fp8-scaled h
    SB = 2.0 ** (-SPL)         # psum -> unscaled bf16 h

    # ---------------- program ----------------
    # in-proj: h = relu(x @ W_in1 + c_in), fp8, K=512 (2 pairs)
    h8 = new_tiles(h8ps, FP8)
    xp = [x8[:, 0:2, :], x8[:, 2:4, :]]
    fp8_layer(Win18, Cin8, xp, 2, h8, S8)

    # fp8 Picard iterations (the last evicts to bf16 for the polish/out)
    hb = None
    for i in range(n8):
        last = i == n8 - 1
        out_tiles = new_tiles(hbps, BF16) if last else new_tiles(h8ps, FP8)
        rhs_pairs = [t[:, :, :] for t in h8]
        fp8_layer(W218, C8, rhs_pairs, 4, out_tiles, SB if last else S8)
        if last:
            hb = out_tiles
        else:
            h8 = out_tiles

    # bf16 polish iterations
    for j in range(nb):
        nxt = new_tiles(hbps, BF16)
        pts = new_psums()
        for n in range(NCD):
            pslice = pts[n // 2][:, n % 2, :]
            nc.tensor.matmul(pslice, lhsT=Cb[:, n, :], rhs=onesb[:, :],
                             start=True, stop=False)
            for c in range(NCD):
                nc.tensor.matmul(pslice, lhsT=W21b[:, c, n, :],
                                 rhs=hb[c // 2][:, c % 2, :], start=False,
                                 stop=(c == NCD - 1))
        evict4(pts, nxt, 1.0, relu=True)
        hb = nxt

    # out-proj: out = h @ W2out + c_out  (bf16 weights, fp32 out)
    ots = new_psums()[:2]
    for o in range(NCO):
        pslice = ots[o // 2][:, o % 2, :]
        nc.tensor.matmul(pslice, lhsT=Coutb[:, o, :], rhs=onesb[:, :],
                         start=True, stop=False)
        for c in range(NCD):
            nc.tensor.matmul(pslice, lhsT=W2outb[:, c, o, :],
                             rhs=hb[c // 2][:, c % 2, :], start=False,
                             stop=(c == NCD - 1))
    outT = [pool.tile([P, 2, B], F32, name=f"outT{j}")
            for j, pool in enumerate(otps)]
    nc.scalar.activation(outT[0][:, :, :], ots[0][:, :, :], AF.Copy,
                         bias=0.0, scale=1.0)
    nc.vector.tensor_scalar(out=outT[1][:, :, :], in0=ots[1][:, :, :],
                            scalar1=1.0, scalar2=None, op0=AL.mult)
    nc.sync.dma_start(out=d_out[:, 0 : 2 * B], in_=outT[0][:, :, :])
    nc.gpsimd.dma_start(out=d_out[:, 2 * B :], in_=outT[1][:, :, :])


def build_program(n8: int = N_FP8, nb: int = N_BF16) -> bass.Bass:
    from contextlib import ExitStack

    from concourse import bacc

    nc = bacc.Bacc(trn_type="TRN2", target_bir_lowering=False)
    with ExitStack() as ctx:
        tc = ctx.enter_context(TileContext(nc))
        _emit(nc, tc, ctx, n8, nb)
    nc.compile()
    return nc


def _fold_weights(inputs):
    """Host-side weight folding in fp64 (cheap: ~3.5 GFLOP once per call)."""
    f64 = np.float64
    W_in, b_in = inputs["W_in"].astype(f64), inputs["b_in"].astype(f64)
    W1, b1 = inputs["W1"].astype(f64), inputs["b1"].astype(f64)
    W2, b2 = inputs["W2"].astype(f64), inputs["b2"].astype(f64)
    W_out, b_out = inputs["W_out"].astype(f64), inputs["b_out"].astype(f64)
    return {
        "W_in1": W_in @ W1, "c_in": b_in @ W1 + b1,
        "W21": W2 @ W1, "c": b2 @ W1 + b1,
        "W2out": W2 @ W_out, "c_out": b2 @ W_out + b_out,
    }


def _pack_w8(W, sw):
    """[K, N] -> [128, K//256, 2, N//128, 128] fp8 (DoubleRow pairs)."""
    K, N = W.shape
    t = (W * sw).astype(fp8)
    t = t.reshape(K // 256, 2, P, N // P, P).transpose(2, 0, 1, 3, 4)
    return np.ascontiguousarray(t.reshape(P, -1))


def _pack_wb(W):
    """[K, N] -> [128, K//128, N//128, 128] bf16."""
    K, N = W.shape
    t = W.astype(bf16).reshape(K // P, P, N // P, P).transpose(1, 0, 2, 3)
    return np.ascontiguousarray(t.reshape(P, -1))


def _pack_bias8(c):
    """[N] -> [128, 2, N//128, 128] fp8: partition 0 = (hi, lo) rows.
    Paired with the (2^7, 2^4) ones vector this contributes c*2^16."""
    hi64 = c * 2.0 ** (SPL - 7)
    hi = hi64.astype(fp8)
    lo = ((hi64 - hi.astype(np.float64)) * 2.0 ** 3).astype(fp8)
    arr = np.zeros((P, 2, c.shape[0] // P, P), fp8)
    arr[0, 0] = hi.reshape(-1, P)
    arr[0, 1] = lo.reshape(-1, P)
    return np.ascontiguousarray(arr.reshape(P, -1))


def _pack_biasb(c):
    arr = np.zeros((P, c.shape[0] // P, P), bf16)
    arr[0] = c.astype(bf16).reshape(-1, P)
    return np.ascontiguousarray(arr.reshape(P, -1))


def _prep_inputs(inputs, nb: int = N_BF16):
    F = _fold_weights(inputs)
    ones8 = np.zeros((P, 2, B), fp8)
    ones8[0, 0, :] = 2.0 ** 7
    ones8[0, 1, :] = 2.0 ** 4
    onesb = np.zeros((P, B), bf16)
    onesb[0, :] = 1.0
    sw = 2.0 ** SWL
    shared = {
        "ones8": np.ascontiguousarray(ones8.reshape(P, -1)),
        "onesb": onesb,
        "win18": _pack_w8(F["W_in1"], sw),
        "cin8": _pack_bias8(F["c_in"]),
        "w218": _pack_w8(F["W21"], sw),
        "c8": _pack_bias8(F["c"]),
        "w2outb": _pack_wb(F["W2out"]),
        "coutb": _pack_biasb(F["c_out"]),
    }
    if nb > 0:
        shared["w21b"] = _pack_wb(F["W21"])
        shared["cb"] = _pack_biasb(F["c"])
    x = inputs["x"]
    in_maps = []
    for cidx in range(N_CORES):
        xs = x[cidx * B : (cidx + 1) * B].astype(np.float64)   # [128, 512]
        x8t = (xs.T * 2.0 ** SZL).astype(fp8)                  # [512, 128]
        x8t = x8t.reshape(NCI, P, B).transpose(1, 0, 2)        # [128, 4, 128]
        im = {"x8": np.ascontiguousarray(x8t.reshape(P, -1))}
        im.update(shared)
        in_maps.append(im)
    return in_maps


_CACHE = {}


def run_on_hw(inputs, n8: int = N_FP8, nb: int = N_BF16, trace: bool = False):
    """Returns (output [1024, 512] fp32, BassKernelResults)."""
    from concourse.bass_utils import run_bass_kernel_spmd

    key = (n8, nb)
    if key not in _CACHE:
        _CACHE[key] = build_program(n8, nb)
    nc = _CACHE[key]
    in_maps = _prep_inputs(inputs, nb)
    res = run_bass_kernel_spmd(nc, in_maps, list(range(N_CORES)), trace=trace)
    outs = []
    for i in range(N_CORES):
        oT = np.asarray(res.results[i]["out"], dtype=np.float32)  # [128, 4*128]
        oT = oT.reshape(P, NCO, B).transpose(2, 1, 0).reshape(B, DOUT)
        outs.append(oT)
    return np.concatenate(outs, axis=0), res


def bench_on_hw(inputs, n8: int = N_FP8, nb: int = N_BF16, reps: int = 32):
    """Per-execution device time via pipelined repeated execution."""
    import time

    import jax
    from jax.sharding import Mesh, PartitionSpec
    from jax.experimental.shard_map import shard_map

    from concourse import bass2jax, mybir as mb

    key = (n8, nb)
    if key not in _CACHE:
        _CACHE[key] = build_program(n8, nb)
    nc = _CACHE[key]
    bass2jax.install_neuronx_cc_hook()

    partition_name = nc.partition_id_tensor.name if nc.partition_id_tensor else None
    in_names, out_names, out_avals, zero_outs = [], [], [], []
    for alloc in nc.m.functions[0].allocations:
        if not isinstance(alloc, mb.MemoryLocationSet):
            continue
        name = alloc.memorylocations[0].name
        if alloc.kind == "ExternalInput":
            if name != partition_name:
                in_names.append(name)
        elif alloc.kind == "ExternalOutput":
            out_names.append(name)
            shape = tuple(alloc.tensor_shape)
            dtype = mb.dt.np(alloc.dtype)
            out_avals.append(jax.core.ShapedArray(shape, dtype))
            zero_outs.append(np.zeros(shape, dtype))
    n_params = len(in_names)
    in_names_all = in_names + out_names
    if partition_name is not None:
        in_names_all.append(partition_name)

    def _body(*args):
        operands = list(args)
        if partition_name is not None:
            operands.append(bass2jax.partition_id_tensor())
        outs = bass2jax._bass_exec_p.bind(
            *operands,
            out_avals=tuple(out_avals),
            in_names=tuple(in_names_all),
            out_names=tuple(out_names),
            lowering_input_output_aliases=(),
            sim_require_finite=True,
            sim_require_nnan=True,
            nc=nc,
        )
        return tuple(outs)

    in_maps = _prep_inputs(inputs, nb)
    devices = jax.devices()[:N_CORES]
    mesh = Mesh(np.asarray(devices), ("core",))
    in_specs = (PartitionSpec("core"),) * (n_params + len(out_names))
    out_specs = (PartitionSpec("core"),) * len(out_names)
    sharded = jax.jit(
        shard_map(_body, mesh=mesh, in_specs=in_specs, out_specs=out_specs,
                  check_rep=False),
        keep_unused=True,
    )
    concat_in = [
        np.concatenate([np.asarray(in_maps[c][nm]) for c in range(N_CORES)], axis=0)
        for nm in in_names
    ]
    concat_zeros = [
        np.zeros((N_CORES * z.shape[0], *z.shape[1:]), z.dtype) for z in zero_outs
    ]
    args = [jax.device_put(a) for a in concat_in + concat_zeros]
    out = sharded(*args)
    jax.block_until_ready(out)
    best = float("inf")
    for _ in range(3):
        t0 = time.perf_counter()
        outs = [sharded(*args) for _ in range(reps)]
        jax.block_until_ready(outs)
        dt = (time.perf_counter() - t0) / reps
        best = min(best, dt)
    out_np = np.asarray(out[0], dtype=np.float32)
    return best, out_np


def kernel(**inputs) -> np.ndarray:
    out, _ = run_on_hw(inputs)
    return out


if __name__ == "__main__":
    nc = build_program()
    print("built ok")


# revision 9
# speedup vs baseline: 2.0933x; 1.1847x over previous
"""DEQ MLP — Trainium2 Bass kernel, v3 (Picard + folded weights + fp8).

Problem: z* = fixpoint of f(z) = relu(z@W1+b1)@W2+b2, z0 = x@W_in+b_in,
out = z*@W_out + b_out.  B=1024, D=1024.  Reference solves with Anderson
acceleration (m=6, 40 iters); but f is strongly contractive (~0.17/iter),
so plain Picard iteration reaches the bf16-precision fixed point in ~7
steps — no Anderson machinery needed.

Key restructurings (validated numerically against an fp64 oracle):
 1. h-space iteration with host-folded weights: substituting z = h@W2+b2
    into h' = relu(z@W1+b1) gives  h' = relu(h@(W2@W1) + (b2@W1+b1)) —
    ONE 1024x1024 matmul per iteration instead of two.  Input/output
    projections fold likewise.
 2. fp8(e4m3) DoubleRow matmuls (K=256/instr, 0.5 PE cycles/row) for the
    first N_FP8 iterations, power-of-2 scales (W*2^11, act*2^5); an
    optional bf16 polish iteration pins the fixed point to the bf16
    floor.  Emulated end-to-end: (6,1) rel err 2.8e-3, (6,0) 4.4e-3
    (gate 2e-2); HW matched emulation within 4%.
 3. Biases ride IN the matmul as an extra contraction pair (hi/lo fp8
    split on partition 0 x ones vector), so PSUM evictions are pure
    relu(psum * 2^-k).
 4. h lives in FOUR separate 2-chunk tiles (one per DoubleRow pair) so
    the 4 eviction instructions (alternating ACT/DVE) have no
    write-after-write ordering between them — they pipeline under the
    PE's next groups instead of serializing (+1.2us/iter otherwise).
 5. Weight DMAs spread across 4 engine queues (sync/gpsimd/vector/
    scalar), big tensors split in half, so the serial-DMA head shrinks
    from ~17us to ~2us of exposed latency.
 6. Pure data parallel: batch 1024 -> 128 rows/core on 8 cores; weights
    replicated; no collectives; all layouts feature-major, zero device
    transposes.
"""

import os
import sys

for _p in ("/opt/trn_rl_repo", "/root/.axon_site/_ro/trn_rl_repo"):
    if os.path.isdir(_p) and _p not in sys.path:
        sys.path.insert(0, _p)

import numpy as np
import ml_dtypes

import concourse.bass as bass
import concourse.mybir as mybir
from concourse.tile import TileContext

BF16 = mybir.dt.bfloat16
FP8 = mybir.dt.float8e4
F32 = mybir.dt.float32
AL = mybir.AluOpType
AF = mybir.ActivationFunctionType
DR = mybir.MatmulPerfMode.DoubleRow

P = 128
D = 1024           # hidden width (h space)
DIN = 512
DOUT = 512
NCD = D // P       # 8
NCI = DIN // P     # 4
NCO = DOUT // P    # 4
NPAIR = NCD // 2   # 4 DoubleRow pairs over the hidden dim
N_CORES = 8
B = 1024 // N_CORES  # 128 batch rows per core

# power-of-2 scales for fp8: weights *2^11, activations *2^5, psum *2^16
SWL, SZL = 11, 5
SPL = SWL + SZL

N_FP8 = 6          # fp8 Picard iterations
N_BF16 = 0         # bf16 polish iterations (fp8-only hits 4.2e-3; gate 2e-2)

bf16 = ml_dtypes.bfloat16
fp8 = ml_dtypes.float8_e4m3


def _emit(nc: bass.Bass, tc, ctx, n8: int, nb: int):
    # ---------------- DRAM I/O ----------------
    def din(name, free, dt):
        return nc.declare_dram_parameter(name, [P, free], dt, isOutput=False)

    d_x8 = din("x8", NCI * B, FP8)
    d_ones8 = din("ones8", 2 * B, FP8)
    d_onesb = din("onesb", B, BF16)
    d_win18 = din("win18", 2 * 2 * NCD * P, FP8)
    d_cin8 = din("cin8", 2 * NCD * P, FP8)
    d_w218 = din("w218", 4 * 2 * NCD * P, FP8)
    d_c8 = din("c8", 2 * NCD * P, FP8)
    if nb > 0:
        d_w21b = din("w21b", NCD * NCD * P, BF16)
        d_cb = din("cb", NCD * P, BF16)
    d_w2outb = din("w2outb", NCD * NCO * P, BF16)
    d_coutb = din("coutb", NCO * P, BF16)
    d_out = nc.declare_dram_parameter("out", [P, NCO * B], F32, isOutput=True)

    consts = ctx.enter_context(tc.tile_pool(name="consts", bufs=1))
    state = ctx.enter_context(tc.tile_pool(name="state", bufs=1))
    # one pool per DoubleRow pair so the 4 evictions have no WAW ordering
    h8ps = [ctx.enter_context(tc.tile_pool(name=f"h8p{j}", bufs=2))
            for j in range(NPAIR)]
    hbps = [ctx.enter_context(tc.tile_pool(name=f"hbp{j}", bufs=2))
            for j in range(NPAIR)]
    otps = [ctx.enter_context(tc.tile_pool(name=f"otp{j}", bufs=1))
            for j in range(2)]
    pps = [ctx.enter_context(tc.tile_pool(name=f"pp{j}", bufs=2, space="PSUM"))
           for j in range(NPAIR)]

    # ---------------- constants into SBUF ----------------
    x8 = consts.tile([P, NCI, B], FP8)
    ones8 = consts.tile([P, 2, B], FP8)
    onesb = consts.tile([P, B], BF16)
    Win18 = consts.tile([P, 2, 2, NCD, P], FP8)
    Cin8 = consts.tile([P, 2, NCD, P], FP8)
    W218 = consts.tile([P, 4, 2, NCD, P], FP8)
    C8 = consts.tile([P, 2, NCD, P], FP8)
    if nb > 0:
        W21b = consts.tile([P, NCD, NCD, P], BF16)
        Cb = consts.tile([P, NCD, P], BF16)
    W2outb = consts.tile([P, NCD, NCO, P], BF16)
    Coutb = consts.tile([P, NCO, P], BF16)

    # DMA queue assignment (only SP/Activation/gpsimd may issue DMAs):
    # three parallel queues, criticals first on each.
    nc.sync.dma_start(out=x8[:, :, :], in_=d_x8[:, :])
    nc.sync.dma_start(out=ones8[:, :, :], in_=d_ones8[:, :])
    nc.gpsimd.dma_start(out=Cin8[:, :, :, :], in_=d_cin8[:, :])
    nc.gpsimd.dma_start(out=Win18[:, :, :, :, :], in_=d_win18[:, :])
    HP = 4 * 2 * NCD * P // 2
    nc.scalar.dma_start(out=W218[:, 0:2, :, :, :], in_=d_w218[:, 0:HP])
    nc.scalar.dma_start(out=W218[:, 2:4, :, :, :], in_=d_w218[:, HP:])
    nc.sync.dma_start(out=C8[:, :, :, :], in_=d_c8[:, :])
    nc.sync.dma_start(out=onesb[:, :], in_=d_onesb[:, :])
    nc.scalar.dma_start(out=W2outb[:, :, :, :], in_=d_w2outb[:, :])
    nc.sync.dma_start(out=Coutb[:, :, :], in_=d_coutb[:, :])
    if nb > 0:
        WP = NCD * NCD * P // 2
        nc.scalar.dma_start(out=W21b[:, 0:NCD // 2, :, :], in_=d_w21b[:, 0:WP])
        nc.gpsimd.dma_start(out=W21b[:, NCD // 2 :, :, :], in_=d_w21b[:, WP:])
        nc.sync.dma_start(out=Cb[:, :, :], in_=d_cb[:, :])

    def evict4(pts, out_tiles, scale, relu):
        """4 independent PSUM tiles -> 4 h tiles, alternating ACT/DVE."""
        for j in range(NPAIR):
            if j % 2 == 0:
                nc.scalar.activation(
                    out_tiles[j][:, :, :], pts[j][:, :, :],
                    AF.Relu if relu else AF.Copy, bias=0.0, scale=scale,
                )
            else:
                nc.vector.tensor_scalar(
                    out=out_tiles[j][:, :, :], in0=pts[j][:, :, :],
                    scalar1=scale, scalar2=0.0, op0=AL.mult, op1=AL.max,
                )

    _pc = [0]

    def new_psums(dt=F32):
        _pc[0] += 1
        return [pool.tile([P, 2, B], dt, name=f"pt{_pc[0]}_{j}", tag=f"pt{j}")
                for j, pool in enumerate(pps)]

    def fp8_layer(Wt, biast, rhs_pairs, npairs, out_tiles, out_scale):
        pts = new_psums()
        for n in range(NCD):
            pslice = pts[n // 2][:, n % 2, :]
            nc.tensor.matmul(pslice, lhsT=biast[:, :, n, :],
                             rhs=ones8[:, :, :], start=True, stop=False,
                             perf_mode=DR)
            for cp in range(npairs):
                nc.tensor.matmul(pslice, lhsT=Wt[:, cp, :, n, :],
                                 rhs=rhs_pairs[cp], start=False,
                                 stop=(cp == npairs - 1), perf_mode=DR)
        evict4(pts, out_tiles, out_scale, relu=True)

    _tc = [0]

    def new_tiles(pools, dt):
        _tc[0] += 1
        return [pool.tile([P, 2, B], dt, name=f"h{_tc[0]}_{j}")
                for j, pool in enumerate(pools)]

    S8 = 2.0 ** (SZL - SPL)    # psum -> fp8-scaled h
    SB = 2.0 ** (-SPL)         # psum -> unscaled bf16 h

    # ---------------- program ----------------
    # in-proj: h = relu(x @ W_in1 + c_in), fp8, K=512 (2 pairs)
    h8 = new_tiles(h8ps, FP8)
    xp = [x8[:, 0:2, :], x8[:, 2:4, :]]
    fp8_layer(Win18, Cin8, xp, 2, h8, S8)

    # fp8 Picard iterations (the last evicts to bf16 for the polish/out)
    hb = None
    for i in range(n8):
        last = i == n8 - 1
        out_tiles = new_tiles(hbps, BF16) if last else new_tiles(h8ps, FP8)
        rhs_pairs = [t[:, :, :] for t in h8]
        fp8_layer(W218, C8, rhs_pairs, 4, out_tiles, SB if last else S8)
        if last:
            hb = out_tiles
        else:
            h8 = out_tiles

    # bf16 polish iterations
    for j in range(nb):
        nxt = new_tiles(hbps, BF16)
        pts = new_psums()
        for n in range(NCD):
            pslice = pts[n // 2][:, n % 2, :]
            nc.tensor.matmul(pslice, lhsT=Cb[:, n, :], rhs=onesb[:, :],
                             start=True, stop=False)
            for c in range(NCD):
                nc.tensor.matmul(pslice, lhsT=W21b[:, c, n, :],
                                 rhs=hb[c // 2][:, c % 2, :], start=False,
                                 stop=(c == NCD - 1))
        evict4(pts, nxt, 1.0, relu=True)
        hb = nxt

    # out-proj: out = h @ W2out + c_out  (bf16 weights, fp32 out)
    ots = new_psums()[:2]
    for o in range(NCO):
        pslice = ots[o // 2][:, o % 2, :]
        nc.tensor.matmul(pslice, lhsT=Coutb[:, o, :], rhs=onesb[:, :],
                         start=True, stop=False)
        for c in range(NCD):
            nc.tensor.matmul(pslice, lhsT=W2outb[:, c, o, :],
                             rhs=hb[c // 2][:, c % 2, :], start=False,
                             stop=(c == NCD - 1))
    outT = [pool.tile([P, 2, B], F32, name=f"outT{j}")
            for j, pool in enumerate(otps)]
    nc.scalar.activation(outT[0][:, :, :], ots[0][:, :, :], AF.Copy,
                         bias=0.0, scale=1.0)
    nc.vector.tensor_scalar(out=outT[1][:, :, :], in0=ots[1][:, :, :],
                            scalar1=1.0, scalar2=None, op0=AL.mult)
    nc.sync.dma_start(out=d_out[:, 0 : 2 * B], in_=outT[0][:, :, :])
    nc.sync.dma_start(out=d_out[:, 2 * B :], in_=outT[1][:, :, :])


def build_program(n8: int = N_FP8, nb: int = N_BF16) -> bass.Bass:
    from contextlib import ExitStack

    from concourse import bacc

    nc = bacc.Bacc(trn_type="TRN2", target_bir_lowering=False)
    with ExitStack() as ctx:
        tc = ctx.enter_context(TileContext(nc))
        _emit(nc, tc, ctx, n8, nb)
    nc.compile()
    return nc


def _fold_weights(inputs):
    """Host-side weight folding in fp64 (cheap: ~3.5 GFLOP once per call)."""
    f64 = np.float64
    W_in, b_in = inputs["W_in"].astype(f64), inputs["b_in"].astype(f64)
    W1, b1 = inputs["W1"].astype(f64), inputs["b1"].astype(f64)
    W2, b2 = inputs["W2"].astype(f64), inputs["b2"].astype(f64)
    W_out, b_out = inputs["W_out"].astype(f64), inputs["b_out"].astype(f64)
    return {
        "W_in1": W_in @ W1, "c_in": b_in @ W1 + b1,
        "W21": W2 @ W1, "c": b2 @ W1 + b1,
        "W2out": W2 @ W_out, "c_out": b2 @ W_out + b_out,
    }


def _pack_w8(W, sw):
    """[K, N] -> [128, K//256, 2, N//128, 128] fp8 (DoubleRow pairs)."""
    K, N = W.shape
    t = (W * sw).astype(fp8)
    t = t.reshape(K // 256, 2, P, N // P, P).transpose(2, 0, 1, 3, 4)
    return np.ascontiguousarray(t.reshape(P, -1))


def _pack_wb(W):
    """[K, N] -> [128, K//128, N//128, 128] bf16."""
    K, N = W.shape
    t = W.astype(bf16).reshape(K // P, P, N // P, P).transpose(1, 0, 2, 3)
    return np.ascontiguousarray(t.reshape(P, -1))


def _pack_bias8(c):
    """[N] -> [128, 2, N//128, 128] fp8: partition 0 = (hi, lo) rows.
    Paired with the (2^7, 2^4) ones vector this contributes c*2^16."""
    hi64 = c * 2.0 ** (SPL - 7)
    hi = hi64.astype(fp8)
    lo = ((hi64 - hi.astype(np.float64)) * 2.0 ** 3).astype(fp8)
    arr = np.zeros((P, 2, c.shape[0] // P, P), fp8)
    arr[0, 0] = hi.reshape(-1, P)
    arr[0, 1] = lo.reshape(-1, P)
    return np.ascontiguousarray(arr.reshape(P, -1))


def _pack_biasb(c):
    arr = np.zeros((P, c.shape[0] // P, P), bf16)
    arr[0] = c.astype(bf16).reshape(-1, P)
    return np.ascontiguousarray(arr.reshape(P, -1))


def _prep_inputs(inputs, nb: int = N_BF16):
    F = _fold_weights(inputs)
    ones8 = np.zeros((P, 2, B), fp8)
    ones8[0, 0, :] = 2.0 ** 7
    ones8[0, 1, :] = 2.0 ** 4
    onesb = np.zeros((P, B), bf16)
    onesb[0, :] = 1.0
    sw = 2.0 ** SWL
    shared = {
        "ones8": np.ascontiguousarray(ones8.reshape(P, -1)),
        "onesb": onesb,
        "win18": _pack_w8(F["W_in1"], sw),
        "cin8": _pack_bias8(F["c_in"]),
        "w218": _pack_w8(F["W21"], sw),
        "c8": _pack_bias8(F["c"]),
        "w2outb": _pack_wb(F["W2out"]),
        "coutb": _pack_biasb(F["c_out"]),
    }
    if nb > 0:
        shared["w21b"] = _pack_wb(F["W21"])
        shared["cb"] = _pack_biasb(F["c"])
    x = inputs["x"]
    in_maps = []
    for cidx in range(N_CORES):
        xs = x[cidx * B : (cidx + 1) * B].astype(np.float64)   # [128, 512]
        x8t = (xs.T * 2.0 ** SZL).astype(fp8)                  # [512, 128]
        x8t = x8t.reshape(NCI, P, B).transpose(1, 0, 2)        # [128, 4, 128]
        im = {"x8": np.ascontiguousarray(x8t.reshape(P, -1))}
        im.update(shared)
        in_maps.append(im)
    return in_maps


_CACHE = {}


def run_on_hw(inputs, n8: int = N_FP8, nb: int = N_BF16, trace: bool = False):
    """Returns (output [1024, 512] fp32, BassKernelResults)."""
    from concourse.bass_utils import run_bass_kernel_spmd

    key = (n8, nb)
    if key not in _CACHE:
        _CACHE[key] = build_program(n8, nb)
    nc = _CACHE[key]
    in_maps = _prep_inputs(inputs, nb)
    res = run_bass_kernel_spmd(nc, in_maps, list(range(N_CORES)), trace=trace)
    outs = []
    for i in range(N_CORES):
        oT = np.asarray(res.results[i]["out"], dtype=np.float32)  # [128, 4*128]
        oT = oT.reshape(P, NCO, B).transpose(2, 1, 0).reshape(B, DOUT)
        outs.append(oT)
    return np.concatenate(outs, axis=0), res


def bench_on_hw(inputs, n8: int = N_FP8, nb: int = N_BF16, reps: int = 32):
    """Per-execution device time via pipelined repeated execution."""
    import time

    import jax
    from jax.sharding import Mesh, PartitionSpec
    from jax.experimental.shard_map import shard_map

    from concourse import bass2jax, mybir as mb

    key = (n8, nb)
    if key not in _CACHE:
        _CACHE[key] = build_program(n8, nb)
    nc = _CACHE[key]
    bass2jax.install_neuronx_cc_hook()

    partition_name = nc.partition_id_tensor.name if nc.partition_id_tensor else None
    in_names, out_names, out_avals, zero_outs = [], [], [], []
    for alloc in nc.m.functions[0].allocations:
        if not isinstance(alloc, mb.MemoryLocationSet):
            continue
        name = alloc.memorylocations[0].name
        if alloc.kind == "ExternalInput":
            if name != partition_name:
                in_names.append(name)
        elif alloc.kind == "ExternalOutput":
            out_names.append(name)
            shape = tuple(alloc.tensor_shape)
            dtype = mb.dt.np(alloc.dtype)
            out_avals.append(jax.core.ShapedArray(shape, dtype))
            zero_outs.append(np.zeros(shape, dtype))
    n_params = len(in_names)
    in_names_all = in_names + out_names
    if partition_name is not None:
        in_names_all.append(partition_name)

    def _body(*args):
        operands = list(args)
        if partition_name is not None:
            operands.append(bass2jax.partition_id_tensor())
        outs = bass2jax._bass_exec_p.bind(
            *operands,
            out_avals=tuple(out_avals),
            in_names=tuple(in_names_all),
            out_names=tuple(out_names),
            lowering_input_output_aliases=(),
            sim_require_finite=True,
            sim_require_nnan=True,
            nc=nc,
        )
        return tuple(outs)

    in_maps = _prep_inputs(inputs, nb)
    devices = jax.devices()[:N_CORES]
    mesh = Mesh(np.asarray(devices), ("core",))
    in_specs = (PartitionSpec("core"),) * (n_params + len(out_names))
    out_specs = (PartitionSpec("core"),) * len(out_names)
    sharded = jax.jit(
        shard_map(_body, mesh=mesh, in_specs=in_specs, out_specs=out_specs,
                  check_rep=False),
        keep_unused=True,
    )
    concat_in = [
        np.concatenate([np.asarray(in_maps[c][nm]) for c in range(N_CORES)], axis=0)
        for nm in in_names
    ]
    concat_zeros = [
        np.zeros((N_CORES * z.shape[0], *z.shape[1:]), z.dtype) for z in zero_outs
    ]
    args = [jax.device_put(a) for a in concat_in + concat_zeros]
    out = sharded(*args)
    jax.block_until_ready(out)
    best = float("inf")
    for _ in range(3):
        t0 = time.perf_counter()
        outs = [sharded(*args) for _ in range(reps)]
        jax.block_until_ready(outs)
        dt = (time.perf_counter() - t0) / reps
        best = min(best, dt)
    out_np = np.asarray(out[0], dtype=np.float32)
    return best, out_np


def kernel(**inputs) -> np.ndarray:
    out, _ = run_on_hw(inputs)
    return out


if __name__ == "__main__":
    nc = build_program()
    print("built ok")
